# revision 17
# baseline (speedup 1.0000x reference)
"""Trainium2 Bass kernel for nn_CAC_42511586296007 (circular-mask max-pool descriptor).

Reference computation (per batch b, channel c):
  v = l2norm_over_c(max_hw(x)) + sum over 153 circular masks m of
      l2norm_over_c(max_hw(x * m))
Masks: center point + per-quadrant rings/circles of integer radius on 28x28.

Decomposition (verified bit-exact vs the reference mask construction):
  - 77 segment maxes (center + ring(q, r), q in 0..3, r in 1..19) + an 'outer'
    segment (r2 > 361, feeds only the full-map max).
  - circle(q, r) = max(center, ring(q, 1..r)) -- prefix max over r.
  - masked outputs clamp at 0; full max = max over all segments.

Mapping: batch is sharded 8 ways (4 per core). Per core, per batch b:
  1. DMA the 8 channel-tiles [128c x 784] of batch b.
  2. Mirror all 4 quadrants onto one 15x15 geometry with 4 strided ScalarE
     copies (negative strides) into M[c, cell(225), q(4), ct(8)]; invalid
     edge slots get -3e38. Now every ring is one fixed cell-set and tiles/
     quadrants sit in the contiguous d-dimension.
  3. GPSIMD ap_gather with d=32 pulls ring-sorted cell blocks (240 indices
     instead of 976*8 -- ap_gather costs ~27ns/index + ~1.75ns/element).
  4. One DVE reduce_max per radius bucket -> seg[c, t, slot] (zero padding).
  5. Prefix max for circles, relu clamp, channel norms via PE matmuls,
     1/(norm+eps), broadcast, multiply-accumulate over the 154 slots.
"""

import numpy as np

_B, _C, _HH, _WW = 32, 1024, 28, 28
_S = _HH * _WW            # 784
_NCORES = 8
_BL = _B // _NCORES       # 4 batches per core
_CT = _C // 128           # 8 channel tiles
_NT = _BL * _CT           # 32 tiles per core
_MAXR = 20
_NSLOT = 154              # full + center + 76 rings + 76 circles
_NSEG = 84                # 76 rings + 4 center + 4 outer
_EPS = 1e-6
_NEG = -3.0e38
_QUADS = [(1, 1), (-1, 1), (1, -1), (-1, -1)]   # (sign_x, sign_y) ref order


def _build_tables():
    ij = np.arange(15)
    I, J = np.meshgrid(ij, ij, indexing="ij")
    RING = np.ceil(np.sqrt(I * I + J * J)).astype(int)

    buckets = []                                   # (cells, segslot base)
    for r in range(1, _MAXR):
        cells = [(i, j) for i in range(15) for j in range(15) if RING[i, j] == r]
        buckets.append((cells, (r - 1) * 4))
    buckets.append(([(0, 0)], 76))                 # center
    cells_out = [(i, j) for i in range(15) for j in range(15) if RING[i, j] >= _MAXR]
    buckets.append((cells_out, 80))                # outer (full-max only)

    # two gather chunks; every bucket even-padded (duplicate a cell) so a
    # single TT-max fold can halve the stream before the strided reduces.
    chunk_of = lambda k: 0 if k < 11 else 1
    chunks = [[], []]
    meta = []                                      # (chunk, off_folded, cnt_folded, segbase)
    for k, (cells, segbase) in enumerate(buckets):
        ci = chunk_of(k)
        cc = [i * 15 + j for (i, j) in cells]
        if len(cc) % 2:
            cc.append(cc[0])
        meta.append((ci, len(chunks[ci]) // 2, len(cc) // 2, segbase))
        chunks[ci].extend(cc)
    assert [len(c) for c in chunks] == [112, 128], [len(c) for c in chunks]
    idx_ws = []
    for ch in chunks:
        a = np.asarray(ch, dtype=np.int16)
        w = a.reshape(len(ch) // 16, 16).T
        idx_ws.append(np.ascontiguousarray(np.tile(w, (8, 1))))  # [128, n//16]
    return meta, idx_ws


_META, _IDXW = _build_tables()
_NC_CACHE = None


def _build_nc():
    import concourse.bacc as bacc
    import concourse.mybir as mybir
    from concourse.tile import TileContext

    f32 = mybir.dt.float32
    i16 = mybir.dt.int16
    AX = mybir.AxisListType
    AF = mybir.ActivationFunctionType
    MAX = mybir.AluOpType.max
    MULT = mybir.AluOpType.mult

    nc = bacc.Bacc("TRN2")
    xs = nc.dram_tensor("xs", [_BL, _C, _S], f32, kind="ExternalInput")
    idx_d = [nc.dram_tensor(f"idxg{ci}", [128, _IDXW[ci].shape[1]], i16,
                            kind="ExternalInput") for ci in range(2)]
    ones128_d = nc.dram_tensor("ones128", [128, 1], f32, kind="ExternalInput")
    ones1_d = nc.dram_tensor("ones1", [1, 128], f32, kind="ExternalInput")
    out_d = nc.dram_tensor("out", [128, _NT], f32, kind="ExternalOutput")

    with TileContext(nc) as tc:
        with (
            tc.tile_pool(name="const", bufs=1) as cpool,
            tc.tile_pool(name="x", bufs=2) as xpool,
            tc.tile_pool(name="g", bufs=3) as gpool,
            tc.tile_pool(name="big", bufs=1) as bpool,
            tc.tile_pool(name="sp", bufs=4) as sppool,
            tc.tile_pool(name="small", bufs=2) as smpool,
            tc.tile_pool(name="psn", bufs=4, space="PSUM") as ppool_n,
            tc.tile_pool(name="pbc", bufs=4, space="PSUM") as ppool_b,
        ):
            xq0 = xpool.tile([128, _CT * _S], f32, tag="xq", name="xq0")
            nc.sync.dma_start(
                out=xq0[:], in_=xs[0].rearrange("(t p) s -> p t s", p=128)
            )
            idx_t = []
            for ci in range(2):
                nci = _IDXW[ci].shape[1]
                it = cpool.tile([128, nci], i16, tag=f"idx{ci}")
                nc.sync.dma_start(out=it[:], in_=idx_d[ci][:])
                idx_t.append(it)
            ones128_t = cpool.tile([128, 1], f32, tag="o128")
            nc.sync.dma_start(out=ones128_t[:], in_=ones128_d[:])
            ones1_t = cpool.tile([1, 128], f32, tag="o1")
            nc.sync.dma_start(out=ones1_t[:], in_=ones1_d[:])

            M_pers = [bpool.tile([128, 225 * 32], f32, tag=f"M{k}", name=f"M{k}")
                      for k in range(2)]
            for k in range(2):
                M_v0 = M_pers[k][:].rearrange(
                    "p (i j q t) -> p i j q t", i=15, j=15, q=4
                )
                nc.gpsimd.memset(M_v0[:, 14, :, 0:2, :], _NEG)
                nc.gpsimd.memset(M_v0[:, :, 14, 0, :], _NEG)
                nc.gpsimd.memset(M_v0[:, :, 14, 2, :], _NEG)
            seg_h = [bpool.tile([128, 16 * _NSEG], f32, tag=f"seg{h}", name=f"seg{h}")
                     for h in range(2)]
            seg_hr = [t[:].rearrange("p (t k) -> p t k", t=16) for t in seg_h]
            vt_h = [bpool.tile([128, 16 * _NSLOT], f32, tag=f"vt{h}", name=f"vt{h}")
                    for h in range(2)]
            vt_hr = [t[:].rearrange("p (t k) -> p t k", t=16) for t in vt_h]
            outv = bpool.tile([128, _NT], f32, tag="outv")

            for b in range(_BL):
                # -- load the 8 channel tiles of batch b --
                if b == 0:
                    xq = xq0
                else:
                    xq = xpool.tile([128, _CT * _S], f32, tag="xq")
                    nc.sync.dma_start(
                        out=xq[:],
                        in_=xs[b].rearrange("(t p) s -> p t s", p=128),
                    )
                xq_v = xq[:].rearrange("p (t a c) -> p t a c", t=_CT, a=_HH)

                # -- mirror quadrants into M[c, cell, q, ct] --
                # (persistent buffers; -3e38 edge slots were set once above)
                M = M_pers[b % 2]
                M_v = M[:].rearrange(
                    "p (i j q t) -> p i j q t", i=15, j=15, q=4
                )
                for qi, (sx, sy) in enumerate(_QUADS):
                    ic = 14 if sy == 1 else 15
                    jc = 14 if sx == 1 else 15
                    src = xq_v[
                        :, :,
                        (slice(14, 14 + ic) if sy == 1 else slice(14, None, -1)),
                        (slice(14, 14 + jc) if sx == 1 else slice(14, None, -1)),
                    ]                                    # [p, t, i, j]
                    if b == 0:
                        eng = (nc.vector.tensor_copy if qi < 2
                               else nc.gpsimd.tensor_copy)
                    else:
                        eng = nc.scalar.copy
                    eng(
                        out=M_v[:, 0:ic, 0:jc, qi, :],
                        in_=src.transpose([0, 2, 3, 1]),
                    )

                # -- gather ring-sorted blocks (d=32), fold pairs, reduce --
                gfs = []
                for ci in range(2):
                    ni = _IDXW[ci].shape[1] * 16
                    g = gpool.tile([128, 128 * 32], f32, tag="g")
                    nc.gpsimd.ap_gather(
                        out_ap=g[:, 0:ni * 32], in_ap=M[:],
                        idxs_ap=idx_t[ci][:],
                        channels=128, num_elems=225, d=32, num_idxs=ni,
                    )
                    gp = g[:, 0:ni * 32].rearrange("p (s2 w) -> p s2 w", w=64)
                    gf = gpool.tile([128, 128 * 32], f32, tag="g")
                    nc.vector.tensor_tensor(
                        out=gf[:, 0:ni * 16].rearrange("p (s2 w) -> p s2 w", w=32),
                        in0=gp[:, :, 0:32], in1=gp[:, :, 32:64], op=MAX,
                    )
                    gfs.append(gf)
                for ci, off, cnt, segbase in _META:
                    blk = gfs[ci][:, off * 32:(off + cnt) * 32].rearrange(
                        "p (s q t) -> p t q s", q=4, t=_CT
                    )
                    nc.vector.reduce_max(
                        out=seg_hr[b // 2][:, (b % 2) * _CT:(b % 2 + 1) * _CT,
                                           segbase:segbase + 4],
                        in_=blk, axis=AX.X,
                    )

            # -- phase 2, per half --
            for h in range(2):
                vt_r = vt_hr[h]
                nc.vector.reduce_max(
                    out=vt_r[:, :, 0:1], in_=seg_hr[h], axis=AX.X
                )
                nc.scalar.activation(
                    out=vt_r[:, :, 1:78], in_=seg_hr[h][:, :, 0:77], func=AF.Relu
                )
                for r in range(1, _MAXR):
                    in0 = (
                        vt_r[:, :, 77:78].broadcast_to((128, 16, 4))
                        if r == 1
                        else vt_r[:, :, 78 + (r - 2) * 4: 78 + (r - 1) * 4]
                    )
                    nc.vector.tensor_tensor(
                        out=vt_r[:, :, 78 + (r - 1) * 4: 78 + r * 4],
                        in0=in0,
                        in1=vt_r[:, :, 1 + (r - 1) * 4: 1 + r * 4],
                        op=MAX,
                    )
                for b in (h * 2, h * 2 + 1):
                    bl = b % 2
                    bs = slice(bl * _CT, (bl + 1) * _CT)
                    sq = sppool.tile([128, _CT * _NSLOT], f32, tag="sp")
                    sq_v = sq[:].rearrange("p (t k) -> p t k", t=_CT)
                    ps = ppool_n.tile([1, _NSLOT], f32, tag="psn")
                    # ring part (slots 0:78) does not depend on the prefix
                    nc.scalar.activation(
                        out=sq_v[:, :, 0:78], in_=vt_hr[h][:, bs, 0:78],
                        func=AF.Square,
                    )
                    for ct in range(_CT):
                        nc.tensor.matmul(
                            ps[:, 0:78], ones128_t[:], sq_v[:, ct, 0:78],
                            start=(ct == 0), stop=(ct == _CT - 1),
                        )
                    nc.scalar.activation(
                        out=sq_v[:, :, 78:154], in_=vt_hr[h][:, bs, 78:154],
                        func=AF.Square,
                    )
                    for ct in range(_CT):
                        nc.tensor.matmul(
                            ps[:, 78:154], ones128_t[:], sq_v[:, ct, 78:154],
                            start=(ct == 0), stop=(ct == _CT - 1),
                        )
                    nrm = smpool.tile([1, _NSLOT], f32, tag="nrm")
                    nc.scalar.activation(out=nrm[:], in_=ps[:], func=AF.Sqrt)
                    inv = smpool.tile([1, _NSLOT], f32, tag="inv")
                    scr = smpool.tile([1, _NSLOT], f32, tag="scr")
                    nc.vector.reciprocal_approx_accurate(
                        out=inv[:], in_=nrm[:], scratch=scr[:]
                    )
                    pb = ppool_b.tile([128, _NSLOT], f32, tag="pbc")
                    nc.tensor.matmul(pb[:], ones1_t[:], inv[:], start=True, stop=True)
                    scr2 = sppool.tile([128, _NSLOT], f32, tag="sp")
                    for ct in range(_CT):
                        nc.vector.scalar_tensor_tensor(
                            out=scr2[:],
                            in0=vt_hr[h][:, bl * _CT + ct, :],
                            scalar=1.0,
                            in1=pb[:],
                            op0=MULT,
                            op1=MULT,
                            accum_out=outv[:, b * _CT + ct:b * _CT + ct + 1],
                        )
            nc.sync.dma_start(out=out_d[:], in_=outv[:])

    nc.finalize()
    return nc


def _get_nc():
    global _NC_CACHE
    if _NC_CACHE is None:
        _NC_CACHE = _build_nc()
    return _NC_CACHE


def _run(x, trace=False):
    from concourse.bass_utils import run_bass_kernel_spmd

    nc = _get_nc()
    x = np.ascontiguousarray(np.asarray(x, dtype=np.float32))
    xs = x.reshape(_NCORES, _BL, _C, _S)
    ones128 = np.ones((128, 1), np.float32)
    ones1 = np.ones((1, 128), np.float32)
    in_maps = [
        {
            "xs": np.ascontiguousarray(xs[c]),
            "idxg0": _IDXW[0], "idxg1": _IDXW[1],
            "ones128": ones128,
            "ones1": ones1,
        }
        for c in range(_NCORES)
    ]
    res = run_bass_kernel_spmd(
        nc, in_maps, core_ids=list(range(_NCORES)), trace=trace
    )
    out = np.empty((_B, _C), np.float32)
    for c in range(_NCORES):
        r = np.asarray(res.results[c]["out"])           # [128, 32]
        rr = r.reshape(128, _BL, _CT)                    # [p, b, ct]
        out[c * _BL:(c + 1) * _BL] = rr.transpose(1, 2, 0).reshape(_BL, _C)
    return out.reshape(_B, _C, 1, 1), res


def kernel(x):
    out, _ = _run(x, trace=False)
    return out


# revision 18
# speedup vs baseline: 1.0577x; 1.0577x over previous
"""Trainium2 Bass kernel for nn_CAC_42511586296007 (circular-mask max-pool descriptor).

Reference computation (per batch b, channel c):
  v = l2norm_over_c(max_hw(x)) + sum over 153 circular masks m of
      l2norm_over_c(max_hw(x * m))
Masks: center point + per-quadrant rings/circles of integer radius on 28x28.

Decomposition (verified bit-exact vs the reference mask construction):
  - 77 segment maxes (center + ring(q, r), q in 0..3, r in 1..19) + an 'outer'
    segment (r2 > 361, feeds only the full-map max).
  - circle(q, r) = max(center, ring(q, 1..r)) -- prefix max over r.
  - masked outputs clamp at 0; full max = max over all segments.

Mapping: batch is sharded 8 ways (4 per core). Per core, per batch b:
  1. DMA the 8 channel-tiles [128c x 784] of batch b.
  2. Mirror all 4 quadrants onto one 15x15 geometry with 4 strided ScalarE
     copies (negative strides) into M[c, cell(225), q(4), ct(8)]; invalid
     edge slots get -3e38. Now every ring is one fixed cell-set and tiles/
     quadrants sit in the contiguous d-dimension.
  3. GPSIMD ap_gather with d=32 pulls ring-sorted cell blocks (240 indices
     instead of 976*8 -- ap_gather costs ~27ns/index + ~1.75ns/element).
  4. One DVE reduce_max per radius bucket -> seg[c, t, slot] (zero padding).
  5. Prefix max for circles, relu clamp, channel norms via PE matmuls,
     1/(norm+eps), broadcast, multiply-accumulate over the 154 slots.
"""

import numpy as np

_B, _C, _HH, _WW = 32, 1024, 28, 28
_S = _HH * _WW            # 784
_NCORES = 8
_BL = _B // _NCORES       # 4 batches per core
_CT = _C // 128           # 8 channel tiles
_NT = _BL * _CT           # 32 tiles per core
_MAXR = 20
_NSLOT = 154              # full + center + 76 rings + 76 circles
_NSEG = 84                # 76 rings + 4 center + 4 outer
_EPS = 1e-6
_NEG = -3.0e38
_QUADS = [(1, 1), (-1, 1), (1, -1), (-1, -1)]   # (sign_x, sign_y) ref order


def _build_tables():
    ij = np.arange(15)
    I, J = np.meshgrid(ij, ij, indexing="ij")
    RING = np.ceil(np.sqrt(I * I + J * J)).astype(int)

    buckets = []                                   # (cells, segslot base)
    for r in range(1, _MAXR):
        cells = [(i, j) for i in range(15) for j in range(15) if RING[i, j] == r]
        buckets.append((cells, (r - 1) * 4))
    buckets.append(([(0, 0)], 76))                 # center
    cells_out = [(i, j) for i in range(15) for j in range(15) if RING[i, j] >= _MAXR]
    buckets.append((cells_out, 80))                # outer (full-max only)

    # two gather chunks; every bucket even-padded (duplicate a cell) so a
    # single TT-max fold can halve the stream before the strided reduces.
    chunk_of = lambda k: 0 if k < 11 else 1
    chunks = [[], []]
    meta = []                                      # (chunk, off_folded, cnt_folded, segbase)
    for k, (cells, segbase) in enumerate(buckets):
        ci = chunk_of(k)
        cc = [i * 15 + j for (i, j) in cells]
        if len(cc) % 2:
            cc.append(cc[0])
        meta.append((ci, len(chunks[ci]) // 2, len(cc) // 2, segbase))
        chunks[ci].extend(cc)
    assert [len(c) for c in chunks] == [112, 128], [len(c) for c in chunks]
    idx_ws = []
    for ch in chunks:
        a = np.asarray(ch, dtype=np.int16)
        w = a.reshape(len(ch) // 16, 16).T
        idx_ws.append(np.ascontiguousarray(np.tile(w, (8, 1))))  # [128, n//16]
    return meta, idx_ws


_META, _IDXW = _build_tables()
_NC_CACHE = None


def _build_nc():
    import concourse.bacc as bacc
    import concourse.mybir as mybir
    from concourse.tile import TileContext

    f32 = mybir.dt.float32
    i16 = mybir.dt.int16
    AX = mybir.AxisListType
    AF = mybir.ActivationFunctionType
    MAX = mybir.AluOpType.max
    MULT = mybir.AluOpType.mult

    nc = bacc.Bacc("TRN2")
    xs = nc.dram_tensor("xs", [_BL, _C, _S], f32, kind="ExternalInput")
    idx_d = [nc.dram_tensor(f"idxg{ci}", [128, _IDXW[ci].shape[1]], i16,
                            kind="ExternalInput") for ci in range(2)]
    ones128_d = nc.dram_tensor("ones128", [128, 1], f32, kind="ExternalInput")
    ones1_d = nc.dram_tensor("ones1", [1, 128], f32, kind="ExternalInput")
    out_d = nc.dram_tensor("out", [128, _NT], f32, kind="ExternalOutput")

    with TileContext(nc) as tc:
        with (
            tc.tile_pool(name="const", bufs=1) as cpool,
            tc.tile_pool(name="x", bufs=2) as xpool,
            tc.tile_pool(name="g", bufs=3) as gpool,
            tc.tile_pool(name="big", bufs=1) as bpool,
            tc.tile_pool(name="sp", bufs=4) as sppool,
            tc.tile_pool(name="small", bufs=2) as smpool,
            tc.tile_pool(name="psn", bufs=4, space="PSUM") as ppool_n,
            tc.tile_pool(name="pbc", bufs=4, space="PSUM") as ppool_b,
        ):
            xq0 = xpool.tile([128, _CT * _S], f32, tag="xq", name="xq0")
            nc.sync.dma_start(
                out=xq0[:], in_=xs[0].rearrange("(t p) s -> p t s", p=128)
            )
            idx_t = []
            for ci in range(2):
                nci = _IDXW[ci].shape[1]
                it = cpool.tile([128, nci], i16, tag=f"idx{ci}")
                nc.sync.dma_start(out=it[:], in_=idx_d[ci][:])
                idx_t.append(it)
            ones128_t = cpool.tile([128, 1], f32, tag="o128")
            nc.sync.dma_start(out=ones128_t[:], in_=ones128_d[:])
            ones1_t = cpool.tile([1, 128], f32, tag="o1")
            nc.sync.dma_start(out=ones1_t[:], in_=ones1_d[:])

            M_pers = [bpool.tile([128, 225 * 32], f32, tag=f"M{k}", name=f"M{k}")
                      for k in range(2)]
            for k in range(2):
                M_v0 = M_pers[k][:].rearrange(
                    "p (i j q t) -> p i j q t", i=15, j=15, q=4
                )
                nc.gpsimd.memset(M_v0[:, 14, :, 0:2, :], _NEG)
                nc.gpsimd.memset(M_v0[:, :, 14, 0, :], _NEG)
                nc.gpsimd.memset(M_v0[:, :, 14, 2, :], _NEG)
            seg_h = [bpool.tile([128, 16 * _NSEG], f32, tag=f"seg{h}", name=f"seg{h}")
                     for h in range(2)]
            seg_hr = [t[:].rearrange("p (t k) -> p t k", t=16) for t in seg_h]
            vt_h = [bpool.tile([128, 16 * _NSLOT], f32, tag=f"vt{h}", name=f"vt{h}")
                    for h in range(2)]
            vt_hr = [t[:].rearrange("p (t k) -> p t k", t=16) for t in vt_h]
            outv = bpool.tile([128, _NT], f32, tag="outv")

            for b in range(_BL):
                # -- load the 8 channel tiles of batch b --
                if b == 0:
                    xq = xq0
                else:
                    xq = xpool.tile([128, _CT * _S], f32, tag="xq")
                    nc.sync.dma_start(
                        out=xq[:],
                        in_=xs[b].rearrange("(t p) s -> p t s", p=128),
                    )
                xq_v = xq[:].rearrange("p (t a c) -> p t a c", t=_CT, a=_HH)

                # -- mirror quadrants into M[c, cell, q, ct] --
                # (persistent buffers; -3e38 edge slots were set once above)
                M = M_pers[b % 2]
                M_v = M[:].rearrange(
                    "p (i j q t) -> p i j q t", i=15, j=15, q=4
                )
                for qi, (sx, sy) in enumerate(_QUADS):
                    ic = 14 if sy == 1 else 15
                    jc = 14 if sx == 1 else 15
                    src = xq_v[
                        :, :,
                        (slice(14, 14 + ic) if sy == 1 else slice(14, None, -1)),
                        (slice(14, 14 + jc) if sx == 1 else slice(14, None, -1)),
                    ]                                    # [p, t, i, j]
                    if b == 0 and qi >= 2:
                        eng = nc.vector.tensor_copy
                    else:
                        eng = nc.scalar.copy
                    eng(
                        out=M_v[:, 0:ic, 0:jc, qi, :],
                        in_=src.transpose([0, 2, 3, 1]),
                    )

                # -- gather ring-sorted blocks (d=32), fold pairs, reduce --
                gfs = []
                for ci in range(2):
                    ni = _IDXW[ci].shape[1] * 16
                    g = gpool.tile([128, 128 * 32], f32, tag="g")
                    nc.gpsimd.ap_gather(
                        out_ap=g[:, 0:ni * 32], in_ap=M[:],
                        idxs_ap=idx_t[ci][:],
                        channels=128, num_elems=225, d=32, num_idxs=ni,
                    )
                    gp = g[:, 0:ni * 32].rearrange("p (s2 w) -> p s2 w", w=64)
                    gf = gpool.tile([128, 128 * 32], f32, tag="g")
                    nc.vector.tensor_tensor(
                        out=gf[:, 0:ni * 16].rearrange("p (s2 w) -> p s2 w", w=32),
                        in0=gp[:, :, 0:32], in1=gp[:, :, 32:64], op=MAX,
                    )
                    gfs.append(gf)
                for ci, off, cnt, segbase in _META:
                    blk = gfs[ci][:, off * 32:(off + cnt) * 32].rearrange(
                        "p (s q t) -> p t q s", q=4, t=_CT
                    )
                    nc.vector.reduce_max(
                        out=seg_hr[b // 2][:, (b % 2) * _CT:(b % 2 + 1) * _CT,
                                           segbase:segbase + 4],
                        in_=blk, axis=AX.X,
                    )

            # -- phase 2, per half --
            for h in range(2):
                vt_r = vt_hr[h]
                nc.vector.reduce_max(
                    out=vt_r[:, :, 0:1], in_=seg_hr[h], axis=AX.X
                )
                nc.scalar.activation(
                    out=vt_r[:, :, 1:78], in_=seg_hr[h][:, :, 0:77], func=AF.Relu
                )
                for r in range(1, _MAXR):
                    in0 = (
                        vt_r[:, :, 77:78].broadcast_to((128, 16, 4))
                        if r == 1
                        else vt_r[:, :, 78 + (r - 2) * 4: 78 + (r - 1) * 4]
                    )
                    nc.vector.tensor_tensor(
                        out=vt_r[:, :, 78 + (r - 1) * 4: 78 + r * 4],
                        in0=in0,
                        in1=vt_r[:, :, 1 + (r - 1) * 4: 1 + r * 4],
                        op=MAX,
                    )
                for b in (h * 2, h * 2 + 1):
                    bl = b % 2
                    bs = slice(bl * _CT, (bl + 1) * _CT)
                    sq = sppool.tile([128, _CT * _NSLOT], f32, tag="sp")
                    sq_v = sq[:].rearrange("p (t k) -> p t k", t=_CT)
                    ps = ppool_n.tile([1, _NSLOT], f32, tag="psn")
                    nc.scalar.activation(
                        out=sq[:],
                        in_=vt_h[h][:, bl * _CT * _NSLOT:(bl + 1) * _CT * _NSLOT],
                        func=AF.Square,
                    )
                    for ct in range(_CT):
                        nc.tensor.matmul(
                            ps[:], ones128_t[:], sq_v[:, ct, :],
                            start=(ct == 0), stop=(ct == _CT - 1),
                        )
                    nrm = smpool.tile([1, _NSLOT], f32, tag="nrm")
                    nc.scalar.activation(out=nrm[:], in_=ps[:], func=AF.Sqrt)
                    inv = smpool.tile([1, _NSLOT], f32, tag="inv")
                    scr = smpool.tile([1, _NSLOT], f32, tag="scr")
                    nc.vector.reciprocal_approx_accurate(
                        out=inv[:], in_=nrm[:], scratch=scr[:]
                    )
                    pb = ppool_b.tile([128, _NSLOT], f32, tag="pbc")
                    nc.tensor.matmul(pb[:], ones1_t[:], inv[:], start=True, stop=True)
                    scr2 = sppool.tile([128, _NSLOT], f32, tag="sp")
                    for ct in range(_CT):
                        nc.vector.scalar_tensor_tensor(
                            out=scr2[:],
                            in0=vt_hr[h][:, bl * _CT + ct, :],
                            scalar=1.0,
                            in1=pb[:],
                            op0=MULT,
                            op1=MULT,
                            accum_out=outv[:, b * _CT + ct:b * _CT + ct + 1],
                        )
            nc.sync.dma_start(out=out_d[:], in_=outv[:])

    nc.finalize()
    return nc


def _get_nc():
    global _NC_CACHE
    if _NC_CACHE is None:
        _NC_CACHE = _build_nc()
    return _NC_CACHE


def _run(x, trace=False):
    from concourse.bass_utils import run_bass_kernel_spmd

    nc = _get_nc()
    x = np.ascontiguousarray(np.asarray(x, dtype=np.float32))
    xs = x.reshape(_NCORES, _BL, _C, _S)
    ones128 = np.ones((128, 1), np.float32)
    ones1 = np.ones((1, 128), np.float32)
    in_maps = [
        {
            "xs": np.ascontiguousarray(xs[c]),
            "idxg0": _IDXW[0], "idxg1": _IDXW[1],
            "ones128": ones128,
            "ones1": ones1,
        }
        for c in range(_NCORES)
    ]
    res = run_bass_kernel_spmd(
        nc, in_maps, core_ids=list(range(_NCORES)), trace=trace
    )
    out = np.empty((_B, _C), np.float32)
    for c in range(_NCORES):
        r = np.asarray(res.results[c]["out"])           # [128, 32]
        rr = r.reshape(128, _BL, _CT)                    # [p, b, ct]
        out[c * _BL:(c + 1) * _BL] = rr.transpose(1, 2, 0).reshape(_BL, _C)
    return out.reshape(_B, _C, 1, 1), res


def kernel(x):
    out, _ = _run(x, trace=False)
    return out
